# revision 6
# baseline (speedup 1.0000x reference)
"""Distributed MHA (+fc+residual+LayerNorm) Bass kernel for 8 TRN2 NeuronCores.

Problem: B=2, L=2048, H=16 heads, d_k=d_v=64, d_model=1024 (f32).
Returns (out, attn) like the reference.

Sharding: core c in [0,8): batch b = c//4, head group r = c%4 -> heads
[4r, 4r+4). Each core computes its 4 heads' attention (writing its slice of
the big attn output), the PV matmul, and a partial fc projection; a
ReduceScatter over each batch's 4 cores sums the fc partials and hands each
core a 512-row quarter for residual+LayerNorm.

Algorithm per core/head (all matmuls in float32r, 1 cycle/row):
  pass 1: S[q,k] blocks = qT.T @ kT -> exp (ACT, scale=1/8, accum_out gives
          row sums Z) -> scale by 1/Z (DVE) -> DMA out as attn.
  pass 2: S'[k,q] = kTaug.T @ qaug where the augmented contraction row holds
          (ones, -8*ln Z[q]); exp(S'/8) = attn^T directly (normalized),
          written as f32r and fed straight into the PV matmul accumulation
          (out^T[dv,q]), avoiding any large on-chip transpose.
  fc: y[q,:] += outT^T @ W_shard per head-chunk (K=64), -> DRAM.
  ReduceScatter(4) -> +residual +b_fc -> LayerNorm -> out rows.
"""
import numpy as np

import concourse.bass as bass
import concourse.tile as tile
from concourse import mybir
from concourse.bass_utils import run_bass_kernel_spmd
from concourse.masks import make_identity

F32 = mybir.dt.float32
F32R = mybir.dt.float32r

B, L, H, DK, DM = 2, 2048, 16, 64, 1024
HL = 4              # heads per core
NB = L // 128       # 16 q/k blocks
LQ = L // 4         # 512 rows per core after reduce-scatter
LN_EPS = 1e-5
GROUPS = [[0, 1, 2, 3], [4, 5, 6, 7]]

# ---------------------------------------------------------------------------
# workarounds for the pinned walrus build (max ONE sync wait per instruction)
# ---------------------------------------------------------------------------
_ScopedClock = bass._bass_rust.ScopedClock


def _patched_drain_and_barrier(self, tick_clock, wait_clock):
    nc = self.nc
    drain_inst = nc.sync.drain()
    wait_clock.add_sem_waits(
        drain_inst.ins, _ScopedClock({None: tick_clock.global_clock})
    )
    si = drain_inst.ins.sync_info
    waits = list(si.on_wait or [])
    if len(waits) > 1:
        si.on_wait = [waits[0]]
        allocated = {h.name: h for h in self.sems.allocated().values()}
        for w in waits[1:]:
            nc.sync.wait_ge(allocated[w.ant_name], w.wait_value)
    nc.all_engine_barrier()
    popped = nc._tile_sem_poison_stack.pop()
    assert popped is self._sem_poison
    nc.clear_and_free_semaphores(list(self.sems.allocated().values()))
    nc.all_engine_barrier()


tile.TileContext._drain_and_barrier = _patched_drain_and_barrier


def _split_multi_waits(nc):
    f = nc.m.functions[0]
    for blk in f.blocks:
        insts = list(blk.instructions)
        out = []
        changed = False
        for inst in insts:
            si = inst.sync_info
            waits = list(si.on_wait) if si and si.on_wait else []
            if len(waits) > 1:
                changed = True
                for w in waits[:-1]:
                    nop = mybir.InstNoOp(
                        name=nc.get_next_instruction_name(), ins=[], outs=[]
                    )
                    nop.engine = inst.engine
                    nop.sync_info = mybir.SyncInfo(on_wait=[w], on_update=[])
                    nc.register_instruction(nop)
                    out.append(nop)
                si.on_wait = [waits[-1]]
            out.append(inst)
        if changed:
            blk.instructions = out


# ---------------------------------------------------------------------------
# device program
# ---------------------------------------------------------------------------
def _build():
    nc = bass.Bass("TRN2", target_bir_lowering=False, num_devices=8)

    qd = nc.declare_dram_parameter("q", [L, HL * DK], F32, isOutput=False)
    kd = nc.declare_dram_parameter("k", [L, HL * DK], F32, isOutput=False)
    vd = nc.declare_dram_parameter("v", [L, HL * DK], F32, isOutput=False)
    wd = nc.declare_dram_parameter("w", [HL * DK, DM], F32, isOutput=False)
    residd = nc.declare_dram_parameter("resid", [LQ, DM], F32, isOutput=False)
    bfcd = nc.declare_dram_parameter("bfc", [1, DM], F32, isOutput=False)
    gammad = nc.declare_dram_parameter("gamma", [1, DM], F32, isOutput=False)
    betad = nc.declare_dram_parameter("beta", [1, DM], F32, isOutput=False)

    attnd = nc.declare_dram_parameter("attn", [HL, L, L], F32, isOutput=True)
    outd = nc.declare_dram_parameter("out", [LQ, DM], F32, isOutput=True)

    y_b = nc.dram_tensor("y_b", [L, DM], F32)
    yr_b = nc.dram_tensor("yr_b", [LQ, DM], F32)

    Exp = mybir.ActivationFunctionType.Exp
    Ln = mybir.ActivationFunctionType.Ln
    Sqrt = mybir.ActivationFunctionType.Sqrt

    with tile.TileContext(nc) as tc:
        with (
            tc.tile_pool(name="const", bufs=1) as constp,
            tc.tile_pool(name="big", bufs=1) as bigp,
            tc.tile_pool(name="qkin", bufs=2) as qkin,
            tc.tile_pool(name="e", bufs=2) as epool,
            tc.tile_pool(name="at", bufs=2) as atpool,
            tc.tile_pool(name="stat", bufs=2) as statp,
            tc.tile_pool(name="y", bufs=2) as ypool,
            tc.tile_pool(name="spsum", bufs=2, space="PSUM") as spool,
            tc.tile_pool(name="pvpsum", bufs=1, space="PSUM") as pvpool,
        ):
            ident = constp.tile([128, 128], F32)
            make_identity(nc, ident)
            eps_t = constp.tile([128, 1], F32)
            nc.vector.memset(eps_t, LN_EPS)

            # fc weights [64, 4, 1024] f32r (cast on DMA), resid, ln params
            w4 = bigp.tile([64, HL, DM], F32R)
            nc.gpsimd.dma_start(
                out=w4, in_=wd.ap().rearrange("(c p) m -> p c m", p=64)
            )
            resid_sb = bigp.tile([128, 4, DM], F32)
            nc.sync.dma_start(
                out=resid_sb, in_=residd.ap().rearrange("(i p) m -> p i m", p=128)
            )

            def bcast_row(src, tag):
                t = constp.tile([128, DM], F32, tag=tag)
                src_ap = src.ap()
                b_ap = bass.AP(
                    tensor=src_ap.tensor,
                    offset=src_ap.offset,
                    ap=[[0, 128], list(src_ap.ap[1])],
                )
                nc.gpsimd.dma_start(out=t, in_=b_ap)
                return t

            gamma_t = bcast_row(gammad, "gamma")
            beta_t = bcast_row(betad, "beta")
            bfc_t = bcast_row(bfcd, "bfc")

            # v as f32r [128, 16, 256]
            v_r = bigp.tile([128, NB, HL * DK], F32R)
            nc.gpsimd.dma_start(
                out=v_r, in_=vd.ap().rearrange("(i p) d -> p i d", p=128)
            )

            # augmented transposed operands (full K=128, zero padded).
            # memset can't write f32r -> fill pad rows via cast-DMA from
            # inline const rows (partition-broadcast stride-0 APs).
            qaug = bigp.tile([128, HL, L], F32R)
            kaug = bigp.tile([128, HL, L], F32R)
            zeros_c = nc.inline_tensor(np.zeros((1, HL * L), np.float32))
            ones_c = nc.inline_tensor(np.ones((1, HL * L), np.float32))

            def pad_fill(dst, src_h, nparts):
                s_ap = src_h.ap()
                b_ap = bass.AP(
                    tensor=s_ap.tensor, offset=0,
                    ap=[[0, nparts], [1, HL * L]])
                nc.gpsimd.dma_start(
                    out=dst.rearrange("p h l -> p (h l)"), in_=b_ap)

            pad_fill(qaug[64:128, :, :], zeros_c, 64)
            pad_fill(kaug[64:128, :, :], zeros_c, 64)
            pad_fill(kaug[64:65, :, :], ones_c, 1)

            # transpose q,k: [128,64] blocks -> qaug/kaug rows 0..63
            for t in range(NB):
                q_t = qkin.tile([128, HL * DK], F32, tag="qk")
                nc.sync.dma_start(out=q_t, in_=qd.ap()[128 * t:128 * (t + 1), :])
                k_t = qkin.tile([128, HL * DK], F32, tag="qk")
                nc.sync.dma_start(out=k_t, in_=kd.ap()[128 * t:128 * (t + 1), :])
                trq = spool.tile([64, 512], F32, tag="s")
                trk = spool.tile([64, 512], F32, tag="s")
                for h in range(HL):
                    nc.tensor.transpose(
                        trq[:, 128 * h:128 * (h + 1)],
                        q_t[:, DK * h:DK * (h + 1)], ident)
                    nc.tensor.transpose(
                        trk[:, 128 * h:128 * (h + 1)],
                        k_t[:, DK * h:DK * (h + 1)], ident)
                # strided copy into [64, h, 128t:128t+128]
                qdst = qaug[0:64, :, 128 * t:128 * (t + 1)]
                kdst = kaug[0:64, :, 128 * t:128 * (t + 1)]
                qsrc = trq.rearrange("p (h c) -> p h c", h=HL)
                ksrc = trk.rearrange("p (h c) -> p h c", h=HL)
                nc.vector.tensor_copy(qdst, qsrc)
                nc.scalar.copy(kdst, ksrc)

            # ---------------- main loop over local heads ----------------
            for h in range(HL):
                rsh = statp.tile([128, 2 * NB], F32, tag="rsh")
                # ---- pass 1: S[q,k], exp, normalize, write attn ----
                for i in range(NB):
                    e_t = epool.tile([128, L], F32, tag="e")
                    for half in range(2):
                        s_ps = spool.tile([128, 1024], F32, tag="s")
                        for c2 in range(2):
                            nc.tensor.matmul(
                                s_ps[:, 512 * c2:512 * (c2 + 1)],
                                qaug[0:64, h, 128 * i:128 * (i + 1)],
                                kaug[0:64, h,
                                     1024 * half + 512 * c2:
                                     1024 * half + 512 * (c2 + 1)],
                                start=True, stop=True)
                        nc.scalar.activation(
                            e_t[:, 1024 * half:1024 * (half + 1)], s_ps,
                            Exp, scale=0.125,
                            accum_out=rsh[:, 2 * i + half:2 * i + half + 1])
                    rs_i = statp.tile([128, 1], F32, tag="rs")
                    nc.vector.tensor_add(
                        rs_i, rsh[:, 2 * i:2 * i + 1], rsh[:, 2 * i + 1:2 * i + 2])
                    recip_i = statp.tile([128, 1], F32, tag="recip")
                    nc.vector.reciprocal(recip_i, rs_i)
                    nc.vector.tensor_scalar_mul(e_t, e_t, recip_i)
                    nc.sync.dma_start(
                        out=attnd.ap()[h, 128 * i:128 * (i + 1), :], in_=e_t)

                # ---- lnZ row for pass 2 ----
                rs16 = statp.tile([128, NB], F32, tag="rs16")
                rshv = rsh.rearrange("p (i two) -> p i two", two=2)
                nc.vector.tensor_add(rs16, rshv[:, :, 0], rshv[:, :, 1])
                ln16 = statp.tile([128, NB], F32, tag="ln16")
                nc.scalar.activation(ln16, rs16, Ln)
                m8 = statp.tile([128, NB], F32, tag="m8")
                nc.vector.tensor_scalar_mul(m8, ln16, -8.0)
                trz = spool.tile([16, 128], F32, tag="s")
                nc.tensor.transpose(trz, m8, ident)
                trzs = statp.tile([16, 128], F32R, tag="trzs")
                nc.vector.tensor_copy(trzs, trz)
                nc.gpsimd.dma_start(
                    out=qaug[64:65, h, :].rearrange("o (j p) -> o j p", j=16),
                    in_=trzs)

                # ---- pass 2: attn^T directly + PV accumulation ----
                pv_ps = pvpool.tile([64, L], F32, tag="pv")
                for j in range(NB):
                    at_t = atpool.tile([128, L], F32R, tag="at")
                    for half in range(2):
                        s2 = spool.tile([128, 1024], F32, tag="s")
                        for c2 in range(2):
                            nc.tensor.matmul(
                                s2[:, 512 * c2:512 * (c2 + 1)],
                                kaug[:, h, 128 * j:128 * (j + 1)],
                                qaug[:, h,
                                     1024 * half + 512 * c2:
                                     1024 * half + 512 * (c2 + 1)],
                                start=True, stop=True)
                        nc.scalar.activation(
                            at_t[:, 1024 * half:1024 * (half + 1)], s2,
                            Exp, scale=0.125)
                    for q4 in range(4):
                        nc.tensor.matmul(
                            pv_ps[:, 512 * q4:512 * (q4 + 1)],
                            v_r[:, j, DK * h:DK * (h + 1)],
                            at_t[:, 512 * q4:512 * (q4 + 1)],
                            start=(j == 0), stop=(j == NB - 1),
                            skip_group_check=True)
                if h == 0:
                    outT = bigp.tile([64, HL, L], F32R)
                nc.scalar.copy(outT[:, h, :], pv_ps)

            # ---------------- fc: y = outT.T @ W (partial) ----------------
            for i in range(NB):
                y_ps = pvpool.tile([128, DM], F32, tag="pv")
                for n2 in range(2):
                    for h4 in range(HL):
                        nc.tensor.matmul(
                            y_ps[:, 512 * n2:512 * (n2 + 1)],
                            outT[:, h4, 128 * i:128 * (i + 1)],
                            w4[:, h4, 512 * n2:512 * (n2 + 1)],
                            start=(h4 == 0), stop=(h4 == HL - 1))
                ytmp = ypool.tile([128, DM], F32, tag="y")
                nc.scalar.copy(ytmp, y_ps)
                nc.sync.dma_start(
                    out=y_b.ap()[128 * i:128 * (i + 1), :], in_=ytmp)

            # ---------------- reduce-scatter over the batch group ----------
            nc.gpsimd.collective_compute(
                "ReduceScatter", mybir.AluOpType.add,
                ins=[y_b.ap().opt()], outs=[yr_b.ap().opt()],
                replica_groups=GROUPS)

            # ---------------- residual + LayerNorm -------------------------
            for t in range(LQ // 128):
                yt = ypool.tile([128, DM], F32, tag="y")
                nc.sync.dma_start(
                    out=yt, in_=yr_b.ap()[128 * t:128 * (t + 1), :])
                nc.vector.tensor_add(yt, yt, resid_sb[:, t, :])
                nc.vector.tensor_add(yt, yt, bfc_t)
                stats = statp.tile([128, 2, 6], F32, tag="bn")
                ytv = yt.rearrange("p (s f) -> p s f", s=2)
                for s in range(2):
                    nc.vector.bn_stats(out=stats[:, s, :], in_=ytv[:, s, :])
                mv = statp.tile([128, 2], F32, tag="mv")
                nc.vector.bn_aggr(out=mv, in_=stats)
                sd = statp.tile([128, 1], F32, tag="sd")
                nc.scalar.activation(sd, mv[:, 1:2], Sqrt, bias=eps_t)
                rstd = statp.tile([128, 1], F32, tag="rstd")
                nc.vector.reciprocal(rstd, sd)
                nc.vector.tensor_scalar(
                    out=yt, in0=yt, scalar1=mv[:, 0:1], scalar2=rstd,
                    op0=mybir.AluOpType.subtract, op1=mybir.AluOpType.mult)
                nc.vector.tensor_mul(yt, yt, gamma_t)
                nc.vector.tensor_add(yt, yt, beta_t)
                nc.sync.dma_start(
                    out=outd.ap()[128 * t:128 * (t + 1), :], in_=yt)

    _split_multi_waits(nc)
    return nc


_NC = None


def _get_nc():
    global _NC
    if _NC is None:
        _NC = _build()
    return _NC


def kernel(q, k, v, W_fc, b_fc, ln_gamma, ln_beta, _trace=False, _trace_kwargs=None):
    q = np.ascontiguousarray(q, np.float32)
    k = np.ascontiguousarray(k, np.float32)
    v = np.ascontiguousarray(v, np.float32)
    W_fc = np.ascontiguousarray(W_fc, np.float32)
    b_fc = np.ascontiguousarray(b_fc, np.float32).reshape(1, DM)
    ln_gamma = np.ascontiguousarray(ln_gamma, np.float32).reshape(1, DM)
    ln_beta = np.ascontiguousarray(ln_beta, np.float32).reshape(1, DM)

    in_maps = []
    for c in range(8):
        b, r = c // 4, c % 4
        in_maps.append({
            "q": q[b][:, 256 * r:256 * (r + 1)],
            "k": k[b][:, 256 * r:256 * (r + 1)],
            "v": v[b][:, 256 * r:256 * (r + 1)],
            "w": W_fc[256 * r:256 * (r + 1), :],
            "resid": q[b][LQ * r:LQ * (r + 1), :],
            "bfc": b_fc, "gamma": ln_gamma, "beta": ln_beta,
        })

    nc = _get_nc()
    res = run_bass_kernel_spmd(
        nc, in_maps, core_ids=list(range(8)),
        trace=_trace, **(_trace_kwargs or {}))

    out = np.empty((B, L, DM), np.float32)
    attn = np.empty((B, H, L, L), np.float32)
    for c in range(8):
        b, r = c // 4, c % 4
        rc = res.results[c]
        attn[b, 4 * r:4 * (r + 1)] = rc["attn"]
        out[b, LQ * r:LQ * (r + 1)] = rc["out"]
    kernel.last_exec_ns = res.exec_time_ns
    return out, attn


# revision 10
# speedup vs baseline: 1.0560x; 1.0560x over previous
"""Distributed MHA (+fc+residual+LayerNorm) Bass kernel for 8 TRN2 NeuronCores.

Problem: B=2, L=2048, H=16 heads, d_k=d_v=64, d_model=1024 (f32).
Returns (out, attn) like the reference.

Sharding: core c in [0,8): batch b = c//4, head group r = c%4 -> heads
[4r, 4r+4). Each core computes its 4 heads' attention (writing its slice of
the big attn output), the PV matmul, and a partial fc projection; a
ReduceScatter over each batch's 4 cores sums the fc partials and hands each
core a 512-row quarter for residual+LayerNorm.

Algorithm per core/head (all matmuls in float32r, 1 cycle/row):
  pass 1: S[q,k] blocks = qT.T @ kT -> exp (ACT, scale=1/8, accum_out gives
          row sums Z) -> scale by 1/Z (DVE) -> DMA out as attn.
  pass 2: S'[k,q] = kTaug.T @ qaug where the augmented contraction row holds
          (ones, -8*ln Z[q]); exp(S'/8) = attn^T directly (normalized),
          written as f32r and fed straight into the PV matmul accumulation
          (out^T[dv,q]), avoiding any large on-chip transpose.
  fc: y[q,:] += outT^T @ W_shard per head-chunk (K=64), -> DRAM.
  ReduceScatter(4) -> +residual +b_fc -> LayerNorm -> out rows.
"""
import numpy as np

import concourse.bass as bass
import concourse.tile as tile
from concourse import mybir
from concourse.bass_utils import run_bass_kernel_spmd
from concourse.masks import make_identity

F32 = mybir.dt.float32
F32R = mybir.dt.float32r

B, L, H, DK, DM = 2, 2048, 16, 64, 1024
HL = 4              # heads per core
NB = L // 128       # 16 q/k blocks
LQ = L // 4         # 512 rows per core after reduce-scatter
LN_EPS = 1e-5
GROUPS = [[0, 1, 2, 3], [4, 5, 6, 7]]

# ---------------------------------------------------------------------------
# workarounds for the pinned walrus build (max ONE sync wait per instruction)
# ---------------------------------------------------------------------------
_ScopedClock = bass._bass_rust.ScopedClock


def _patched_drain_and_barrier(self, tick_clock, wait_clock):
    nc = self.nc
    drain_inst = nc.sync.drain()
    wait_clock.add_sem_waits(
        drain_inst.ins, _ScopedClock({None: tick_clock.global_clock})
    )
    si = drain_inst.ins.sync_info
    waits = list(si.on_wait or [])
    if len(waits) > 1:
        si.on_wait = [waits[0]]
        allocated = {h.name: h for h in self.sems.allocated().values()}
        for w in waits[1:]:
            nc.sync.wait_ge(allocated[w.ant_name], w.wait_value)
    nc.all_engine_barrier()
    popped = nc._tile_sem_poison_stack.pop()
    assert popped is self._sem_poison
    nc.clear_and_free_semaphores(list(self.sems.allocated().values()))
    nc.all_engine_barrier()


tile.TileContext._drain_and_barrier = _patched_drain_and_barrier


def _split_multi_waits(nc):
    f = nc.m.functions[0]
    for blk in f.blocks:
        insts = list(blk.instructions)
        out = []
        changed = False
        for inst in insts:
            si = inst.sync_info
            waits = list(si.on_wait) if si and si.on_wait else []
            if len(waits) > 1:
                changed = True
                for w in waits[:-1]:
                    nop = mybir.InstNoOp(
                        name=nc.get_next_instruction_name(), ins=[], outs=[]
                    )
                    nop.engine = inst.engine
                    nop.sync_info = mybir.SyncInfo(on_wait=[w], on_update=[])
                    nc.register_instruction(nop)
                    out.append(nop)
                si.on_wait = [waits[-1]]
            out.append(inst)
        if changed:
            blk.instructions = out


# ---------------------------------------------------------------------------
# device program
# ---------------------------------------------------------------------------
def _build():
    nc = bass.Bass("TRN2", target_bir_lowering=False, num_devices=8)

    qd = nc.declare_dram_parameter("q", [L, HL * DK], F32, isOutput=False)
    kd = nc.declare_dram_parameter("k", [L, HL * DK], F32, isOutput=False)
    vd = nc.declare_dram_parameter("v", [L, HL * DK], F32, isOutput=False)
    wd = nc.declare_dram_parameter("w", [HL * DK, DM], F32, isOutput=False)
    residd = nc.declare_dram_parameter("resid", [LQ, DM], F32, isOutput=False)
    bfcd = nc.declare_dram_parameter("bfc", [1, DM], F32, isOutput=False)
    gammad = nc.declare_dram_parameter("gamma", [1, DM], F32, isOutput=False)
    betad = nc.declare_dram_parameter("beta", [1, DM], F32, isOutput=False)

    attnd = nc.declare_dram_parameter("attn", [HL, L, L], F32, isOutput=True)
    outd = nc.declare_dram_parameter("out", [LQ, DM], F32, isOutput=True)

    y_b = nc.dram_tensor("y_b", [L, DM], F32)
    yr_b = nc.dram_tensor("yr_b", [LQ, DM], F32)

    Exp = mybir.ActivationFunctionType.Exp
    Ln = mybir.ActivationFunctionType.Ln
    Sqrt = mybir.ActivationFunctionType.Sqrt

    with tile.TileContext(nc) as tc:
        with (
            tc.tile_pool(name="const", bufs=1) as constp,
            tc.tile_pool(name="big", bufs=1) as bigp,
            tc.tile_pool(name="qkin", bufs=2) as qkin,
            tc.tile_pool(name="e", bufs=2) as epool,
            tc.tile_pool(name="at", bufs=2) as atpool,
            tc.tile_pool(name="stat", bufs=2) as statp,
            tc.tile_pool(name="y", bufs=2) as ypool,
            tc.tile_pool(name="spsum", bufs=2, space="PSUM") as spool,
            tc.tile_pool(name="pvpsum", bufs=1, space="PSUM") as pvpool,
        ):
            ident = constp.tile([128, 128], F32)
            make_identity(nc, ident)
            eps_t = constp.tile([128, 1], F32)
            nc.vector.memset(eps_t, LN_EPS)

            # fc weights [64, 4, 1024] f32r (cast on DMA), resid, ln params
            w4 = bigp.tile([64, HL, DM], F32R)
            nc.gpsimd.dma_start(
                out=w4, in_=wd.ap().rearrange("(c p) m -> p c m", p=64)
            )
            resid_sb = bigp.tile([128, 4, DM], F32)
            nc.sync.dma_start(
                out=resid_sb, in_=residd.ap().rearrange("(i p) m -> p i m", p=128)
            )

            def bcast_row(src, tag):
                t = constp.tile([128, DM], F32, tag=tag)
                src_ap = src.ap()
                b_ap = bass.AP(
                    tensor=src_ap.tensor,
                    offset=src_ap.offset,
                    ap=[[0, 128], list(src_ap.ap[1])],
                )
                nc.gpsimd.dma_start(out=t, in_=b_ap)
                return t

            gamma_t = bcast_row(gammad, "gamma")
            beta_t = bcast_row(betad, "beta")
            bfc_t = bcast_row(bfcd, "bfc")

            # v as f32r [128, 16, 256]
            v_r = bigp.tile([128, NB, HL * DK], F32R)
            nc.gpsimd.dma_start(
                out=v_r, in_=vd.ap().rearrange("(i p) d -> p i d", p=128)
            )

            # augmented transposed operands (full K=128, zero padded).
            # memset can't write f32r -> fill pad rows via cast-DMA from
            # inline const rows (partition-broadcast stride-0 APs).
            qaug = bigp.tile([128, HL, L], F32R)
            kaug = bigp.tile([128, HL, L], F32R)
            zeros_c = nc.inline_tensor(np.zeros((1, HL * L), np.float32))
            ones_c = nc.inline_tensor(np.ones((1, HL * L), np.float32))

            def pad_fill(dst, src_h, nparts):
                s_ap = src_h.ap()
                b_ap = bass.AP(
                    tensor=s_ap.tensor, offset=0,
                    ap=[[0, nparts], [1, HL * L]])
                nc.gpsimd.dma_start(
                    out=dst.rearrange("p h l -> p (h l)"), in_=b_ap)

            pad_fill(qaug[64:128, :, :], zeros_c, 64)
            pad_fill(kaug[64:128, :, :], zeros_c, 64)
            pad_fill(kaug[64:65, :, :], ones_c, 1)

            # transpose q,k: [128,64] blocks -> qaug/kaug rows 0..63
            for t in range(NB):
                q_t = qkin.tile([128, HL * DK], F32, tag="qk")
                nc.sync.dma_start(out=q_t, in_=qd.ap()[128 * t:128 * (t + 1), :])
                k_t = qkin.tile([128, HL * DK], F32, tag="qk")
                nc.sync.dma_start(out=k_t, in_=kd.ap()[128 * t:128 * (t + 1), :])
                trq = spool.tile([64, 512], F32, tag="s")
                trk = spool.tile([64, 512], F32, tag="s")
                for h in range(HL):
                    nc.tensor.transpose(
                        trq[:, 128 * h:128 * (h + 1)],
                        q_t[:, DK * h:DK * (h + 1)], ident)
                    nc.tensor.transpose(
                        trk[:, 128 * h:128 * (h + 1)],
                        k_t[:, DK * h:DK * (h + 1)], ident)
                # strided copy into [64, h, 128t:128t+128]
                qdst = qaug[0:64, :, 128 * t:128 * (t + 1)]
                kdst = kaug[0:64, :, 128 * t:128 * (t + 1)]
                qsrc = trq.rearrange("p (h c) -> p h c", h=HL)
                ksrc = trk.rearrange("p (h c) -> p h c", h=HL)
                nc.vector.tensor_copy(qdst, qsrc)
                nc.vector.tensor_copy(kdst, ksrc)

            outT = bigp.tile([64, HL, L], F32R)

            # ------------- pipelined main loop over local heads -------------
            # phase h interleaves pass-1 of head h with pass-2 of head h-1 so
            # the PE always has dense work (keeps HAM warm) while ACT chews
            # through the exp stream.
            def p1_block(h, i, rsh):
                e_t = epool.tile([128, L], F32, tag="e")
                for half in range(2):
                    s_ps = spool.tile([128, 1024], F32, tag="s")
                    for c2 in range(2):
                        nc.tensor.matmul(
                            s_ps[:, 512 * c2:512 * (c2 + 1)],
                            qaug[0:64, h, 128 * i:128 * (i + 1)],
                            kaug[0:64, h,
                                 1024 * half + 512 * c2:
                                 1024 * half + 512 * (c2 + 1)],
                            start=True, stop=True)
                    nc.scalar.activation(
                        e_t[:, 1024 * half:1024 * (half + 1)], s_ps,
                        Exp, scale=0.125,
                        accum_out=rsh[:, 2 * i + half:2 * i + half + 1])
                rs_i = statp.tile([128, 1], F32, tag="rs")
                nc.vector.tensor_add(
                    rs_i, rsh[:, 2 * i:2 * i + 1], rsh[:, 2 * i + 1:2 * i + 2])
                recip_i = statp.tile([128, 1], F32, tag="recip")
                nc.vector.reciprocal(recip_i, rs_i)
                nc.vector.tensor_scalar_mul(e_t, e_t, recip_i)
                nc.sync.dma_start(
                    out=attnd.ap()[h, 128 * i:128 * (i + 1), :], in_=e_t)

            def lnz_fill(h, rsh):
                rs16 = statp.tile([128, NB], F32, tag="rs16")
                rshv = rsh.rearrange("p (i two) -> p i two", two=2)
                nc.vector.tensor_add(rs16, rshv[:, :, 0], rshv[:, :, 1])
                ln16 = statp.tile([128, NB], F32, tag="ln16")
                nc.scalar.activation(ln16, rs16, Ln)
                m8 = statp.tile([128, NB], F32, tag="m8")
                nc.vector.tensor_scalar_mul(m8, ln16, -8.0)
                trz = spool.tile([16, 128], F32, tag="s")
                nc.tensor.transpose(trz, m8, ident)
                trzs = statp.tile([16, 128], F32R, tag="trzs")
                nc.vector.tensor_copy(trzs, trz)
                nc.gpsimd.dma_start(
                    out=qaug[64:65, h, :].rearrange("o (j p) -> o j p", j=16),
                    in_=trzs)

            def p2_block(h, j, pv_ps):
                at_t = atpool.tile([128, L], F32R, tag="at")
                for half in range(2):
                    s2 = spool.tile([128, 1024], F32, tag="s")
                    for c2 in range(2):
                        nc.tensor.matmul(
                            s2[:, 512 * c2:512 * (c2 + 1)],
                            kaug[:, h, 128 * j:128 * (j + 1)],
                            qaug[:, h,
                                 1024 * half + 512 * c2:
                                 1024 * half + 512 * (c2 + 1)],
                            start=True, stop=True)
                    nc.scalar.activation(
                        at_t[:, 1024 * half:1024 * (half + 1)], s2,
                        Exp, scale=0.125)
                for q4 in range(4):
                    nc.tensor.matmul(
                        pv_ps[:, 512 * q4:512 * (q4 + 1)],
                        v_r[:, j, DK * h:DK * (h + 1)],
                        at_t[:, 512 * q4:512 * (q4 + 1)],
                        start=(j == 0), stop=(j == NB - 1),
                        skip_group_check=True)

            rshs = {}
            for phase in range(HL + 1):
                h1 = phase            # pass-1 head
                h2 = phase - 1        # pass-2 head
                if h2 >= 0:
                    pv_ps = pvpool.tile([64, L], F32, tag="pv")
                if h1 < HL:
                    rshs[h1] = statp.tile(
                        [128, 2 * NB], F32, tag="rsh", name=f"rsh{h1}")
                for step in range(NB):
                    if h1 < HL:
                        p1_block(h1, step, rshs[h1])
                    if h2 >= 0:
                        p2_block(h2, step, pv_ps)
                if h1 < HL:
                    lnz_fill(h1, rshs[h1])
                if h2 >= 0:
                    nc.vector.tensor_copy(outT[:, h2, :], pv_ps)

            # ---------------- fc: y = outT.T @ W (partial) ----------------
            for i in range(NB):
                y_ps = spool.tile([128, DM], F32, tag="s")
                for n2 in range(2):
                    for h4 in range(HL):
                        nc.tensor.matmul(
                            y_ps[:, 512 * n2:512 * (n2 + 1)],
                            outT[:, h4, 128 * i:128 * (i + 1)],
                            w4[:, h4, 512 * n2:512 * (n2 + 1)],
                            start=(h4 == 0), stop=(h4 == HL - 1))
                ytmp = ypool.tile([128, DM], F32, tag="y")
                nc.vector.tensor_copy(ytmp, y_ps)
                nc.sync.dma_start(
                    out=y_b.ap()[128 * i:128 * (i + 1), :], in_=ytmp)

            # ---------------- reduce-scatter over the batch group ----------
            nc.gpsimd.collective_compute(
                "ReduceScatter", mybir.AluOpType.add,
                ins=[y_b.ap().opt()], outs=[yr_b.ap().opt()],
                replica_groups=GROUPS)

            # ---------------- residual + LayerNorm -------------------------
            for t in range(LQ // 128):
                yt = ypool.tile([128, DM], F32, tag="y")
                nc.sync.dma_start(
                    out=yt, in_=yr_b.ap()[128 * t:128 * (t + 1), :])
                nc.vector.tensor_add(yt, yt, resid_sb[:, t, :])
                nc.vector.tensor_add(yt, yt, bfc_t)
                stats = statp.tile([128, 2, 6], F32, tag="bn")
                ytv = yt.rearrange("p (s f) -> p s f", s=2)
                for s in range(2):
                    nc.vector.bn_stats(out=stats[:, s, :], in_=ytv[:, s, :])
                mv = statp.tile([128, 2], F32, tag="mv")
                nc.vector.bn_aggr(out=mv, in_=stats)
                sd = statp.tile([128, 1], F32, tag="sd")
                nc.scalar.activation(sd, mv[:, 1:2], Sqrt, bias=eps_t)
                rstd = statp.tile([128, 1], F32, tag="rstd")
                nc.vector.reciprocal(rstd, sd)
                nc.vector.tensor_scalar(
                    out=yt, in0=yt, scalar1=mv[:, 0:1], scalar2=rstd,
                    op0=mybir.AluOpType.subtract, op1=mybir.AluOpType.mult)
                nc.vector.tensor_mul(yt, yt, gamma_t)
                nc.vector.tensor_add(yt, yt, beta_t)
                nc.sync.dma_start(
                    out=outd.ap()[128 * t:128 * (t + 1), :], in_=yt)

    _split_multi_waits(nc)
    return nc


_NC = None


def _get_nc():
    global _NC
    if _NC is None:
        _NC = _build()
    return _NC


def kernel(q, k, v, W_fc, b_fc, ln_gamma, ln_beta, _trace=False, _trace_kwargs=None):
    q = np.ascontiguousarray(q, np.float32)
    k = np.ascontiguousarray(k, np.float32)
    v = np.ascontiguousarray(v, np.float32)
    W_fc = np.ascontiguousarray(W_fc, np.float32)
    b_fc = np.ascontiguousarray(b_fc, np.float32).reshape(1, DM)
    ln_gamma = np.ascontiguousarray(ln_gamma, np.float32).reshape(1, DM)
    ln_beta = np.ascontiguousarray(ln_beta, np.float32).reshape(1, DM)

    in_maps = []
    for c in range(8):
        b, r = c // 4, c % 4
        in_maps.append({
            "q": q[b][:, 256 * r:256 * (r + 1)],
            "k": k[b][:, 256 * r:256 * (r + 1)],
            "v": v[b][:, 256 * r:256 * (r + 1)],
            "w": W_fc[256 * r:256 * (r + 1), :],
            "resid": q[b][LQ * r:LQ * (r + 1), :],
            "bfc": b_fc, "gamma": ln_gamma, "beta": ln_beta,
        })

    nc = _get_nc()
    res = run_bass_kernel_spmd(
        nc, in_maps, core_ids=list(range(8)),
        trace=_trace, **(_trace_kwargs or {}))

    out = np.empty((B, L, DM), np.float32)
    attn = np.empty((B, H, L, L), np.float32)
    for c in range(8):
        b, r = c // 4, c % 4
        rc = res.results[c]
        attn[b, 4 * r:4 * (r + 1)] = rc["attn"]
        out[b, LQ * r:LQ * (r + 1)] = rc["out"]
    kernel.last_exec_ns = res.exec_time_ns
    return out, attn


# revision 13
# speedup vs baseline: 1.3804x; 1.3072x over previous
"""Distributed MHA (+fc+residual+LayerNorm) Bass kernel for 8 TRN2 NeuronCores.

Problem: B=2, L=2048, H=16 heads, d_k=d_v=64, d_model=1024 (f32).
Returns (out, attn) like the reference.

Sharding: core c in [0,8): batch b = c//4, head group r = c%4 -> heads
[4r, 4r+4). Each core computes its 4 heads' attention (writing its slice of
the big attn output), the PV matmul, and a partial fc projection; two
chunked ReduceScatters over each batch's 4 cores sum the fc partials and
hand each core 2x256 rows for residual+LayerNorm.

Algorithm per core/head (all matmuls in float32r, 1 cycle/row):
  pass 1: S[q,k] blocks = qT.T @ kT -> exp (ACT, scale=1/8, accum_out gives
          row sums Z) -> scale by 1/Z (DVE) -> DMA out as attn.
  pass 2: S'[k,q] = kaug.T @ qaug where the augmented contraction row holds
          (ones, -8*ln Z[q]); exp(S'/8) = attn^T directly (normalized),
          written as f32r and fed straight into the PV matmul accumulation
          (out^T[dv,q]), avoiding any large on-chip transpose.
Pass 1 of head h is interleaved with pass 2 of head h-1 to keep the PE
dense; fc runs in two column waves so the first ReduceScatter overlaps the
remaining compute.
"""
import numpy as np

import concourse.bass as bass
import concourse.bass_utils as _bu
import concourse.tile as tile
from concourse import mybir
from concourse.bass_utils import run_bass_kernel_spmd
from concourse.masks import make_identity

F32 = mybir.dt.float32
F32R = mybir.dt.float32r

B, L, H, DK, DM = 2, 2048, 16, 64, 1024
HL = 4              # heads per core
NB = L // 128       # 16 q/k blocks
LQ = L // 4         # 512 rows per core after the reduce-scatters
LW = L // 2         # 1024 rows per fc wave
LN_EPS = 1e-5
GROUPS = [[0, 1, 2, 3], [4, 5, 6, 7]]

# ---------------------------------------------------------------------------
# workarounds for the pinned walrus build (max ONE sync wait per instruction)
# + repeated-LDWEIGHTS elision (the default cmdline disables ldw-opt)
# ---------------------------------------------------------------------------
_ScopedClock = bass._bass_rust.ScopedClock


def _patched_drain_and_barrier(self, tick_clock, wait_clock):
    nc = self.nc
    drain_inst = nc.sync.drain()
    wait_clock.add_sem_waits(
        drain_inst.ins, _ScopedClock({None: tick_clock.global_clock})
    )
    si = drain_inst.ins.sync_info
    waits = list(si.on_wait or [])
    if len(waits) > 1:
        si.on_wait = [waits[0]]
        allocated = {h.name: h for h in self.sems.allocated().values()}
        for w in waits[1:]:
            nc.sync.wait_ge(allocated[w.ant_name], w.wait_value)
    nc.all_engine_barrier()
    popped = nc._tile_sem_poison_stack.pop()
    assert popped is self._sem_poison
    nc.clear_and_free_semaphores(list(self.sems.allocated().values()))
    nc.all_engine_barrier()


tile.TileContext._drain_and_barrier = _patched_drain_and_barrier

if not getattr(_bu, "_ldwopt_patched", False):
    _orig_run_command = _bu.run_command

    def _run_command_ldwopt(cmd, **kw):
        cmd = ["--enable-ldw-opt=true" if c == "--enable-ldw-opt=false" else c
               for c in cmd]
        return _orig_run_command(cmd, **kw)

    _bu.run_command = _run_command_ldwopt
    _bu._ldwopt_patched = True


def _split_multi_waits(nc):
    f = nc.m.functions[0]
    for blk in f.blocks:
        insts = list(blk.instructions)
        out = []
        changed = False
        for inst in insts:
            si = inst.sync_info
            waits = list(si.on_wait) if si and si.on_wait else []
            if len(waits) > 1:
                changed = True
                for w in waits[:-1]:
                    nop = mybir.InstNoOp(
                        name=nc.get_next_instruction_name(), ins=[], outs=[]
                    )
                    nop.engine = inst.engine
                    nop.sync_info = mybir.SyncInfo(on_wait=[w], on_update=[])
                    nc.register_instruction(nop)
                    out.append(nop)
                si.on_wait = [waits[-1]]
            out.append(inst)
        if changed:
            blk.instructions = out


# ---------------------------------------------------------------------------
# device program
# ---------------------------------------------------------------------------
def _build():
    nc = bass.Bass("TRN2", target_bir_lowering=False, num_devices=8)

    qd = nc.declare_dram_parameter("q", [L, HL * DK], F32, isOutput=False)
    kd = nc.declare_dram_parameter("k", [L, HL * DK], F32, isOutput=False)
    vd = nc.declare_dram_parameter("v", [L, HL * DK], F32, isOutput=False)
    wd = nc.declare_dram_parameter("w", [HL * DK, DM], F32, isOutput=False)
    residd = nc.declare_dram_parameter("resid", [LQ, DM], F32, isOutput=False)
    bfcd = nc.declare_dram_parameter("bfc", [1, DM], F32, isOutput=False)
    gammad = nc.declare_dram_parameter("gamma", [1, DM], F32, isOutput=False)
    betad = nc.declare_dram_parameter("beta", [1, DM], F32, isOutput=False)

    attnd = nc.declare_dram_parameter("attn", [HL, L, L], F32, isOutput=True)
    outd = nc.declare_dram_parameter("out", [LQ, DM], F32, isOutput=True)

    y_bs = [nc.dram_tensor(f"y{w}_b", [LW, DM], F32) for w in range(2)]
    yr_bs = [nc.dram_tensor(f"yr{w}_b", [LW // 4, DM], F32) for w in range(2)]

    Exp = mybir.ActivationFunctionType.Exp
    Ln = mybir.ActivationFunctionType.Ln
    Sqrt = mybir.ActivationFunctionType.Sqrt

    with tile.TileContext(nc) as tc:
        with (
            tc.tile_pool(name="const", bufs=1) as constp,
            tc.tile_pool(name="big", bufs=1) as bigp,
            tc.tile_pool(name="qkin", bufs=3) as qkin,
            tc.tile_pool(name="e", bufs=3) as epool,
            tc.tile_pool(name="at", bufs=3) as atpool,
            tc.tile_pool(name="stat", bufs=2) as statp,
            tc.tile_pool(name="y", bufs=2) as ypool,
            tc.tile_pool(name="spsum", bufs=3, space="PSUM") as spool,
            tc.tile_pool(name="pvpsum", bufs=2, space="PSUM") as pvpool,
        ):
            ident = constp.tile([128, 128], F32)
            make_identity(nc, ident)
            eps_t = constp.tile([128, 1], F32)
            nc.vector.memset(eps_t, LN_EPS)

            w4 = bigp.tile([64, HL, DM], F32R)
            nc.gpsimd.dma_start(
                out=w4, in_=wd.ap().rearrange("(c p) m -> p c m", p=64)
            )
            resid_sb = bigp.tile([128, 4, DM], F32)
            nc.sync.dma_start(
                out=resid_sb, in_=residd.ap().rearrange("(i p) m -> p i m", p=128)
            )

            def bcast_row(src, tag):
                t = constp.tile([128, DM], F32, tag=tag, name=tag)
                src_ap = src.ap()
                b_ap = bass.AP(
                    tensor=src_ap.tensor,
                    offset=src_ap.offset,
                    ap=[[0, 128], list(src_ap.ap[1])],
                )
                nc.gpsimd.dma_start(out=t, in_=b_ap)
                return t

            gamma_t = bcast_row(gammad, "gamma")
            beta_t = bcast_row(betad, "beta")
            bfc_t = bcast_row(bfcd, "bfc")

            v_r = bigp.tile([128, NB, HL * DK], F32R)

            # augmented transposed operands (full K=128, zero padded).
            # memset can't write f32r -> fill pad rows via cast-DMA from
            # inline const rows (partition-broadcast stride-0 APs).
            qaug = bigp.tile([128, HL, L], F32R)
            kaug = bigp.tile([128, HL, L], F32R)
            zeros_c = nc.inline_tensor(np.zeros((1, HL * L), np.float32))
            ones_c = nc.inline_tensor(np.ones((1, HL * L), np.float32))

            def pad_fill(dst, src_h, nparts):
                s_ap = src_h.ap()
                b_ap = bass.AP(
                    tensor=s_ap.tensor, offset=0,
                    ap=[[0, nparts], [1, HL * L]])
                nc.gpsimd.dma_start(
                    out=dst.rearrange("p h l -> p (h l)"), in_=b_ap)

            pad_fill(qaug[64:128, :, :], zeros_c, 64)
            pad_fill(kaug[64:128, :, :], zeros_c, 64)
            pad_fill(kaug[64:65, :, :], ones_c, 1)

            outT = bigp.tile([64, HL, L], F32R)

            # k transposed up front (pass 1 of head 0 needs all of kT)
            for t in range(NB):
                k_t = qkin.tile([128, HL * DK], F32, tag="qk")
                nc.sync.dma_start(out=k_t, in_=kd.ap()[128 * t:128 * (t + 1), :])
                trk = spool.tile([64, 512], F32, tag="s")
                for h in range(HL):
                    nc.tensor.transpose(
                        trk[:, 128 * h:128 * (h + 1)],
                        k_t[:, DK * h:DK * (h + 1)], ident)
                nc.vector.tensor_copy(
                    kaug[0:64, :, 128 * t:128 * (t + 1)],
                    trk.rearrange("p (h c) -> p h c", h=HL))

            def q_setup_step(t):
                q_t = qkin.tile([128, HL * DK], F32, tag="qk")
                nc.sync.dma_start(out=q_t, in_=qd.ap()[128 * t:128 * (t + 1), :])
                trq = spool.tile([64, 512], F32, tag="s")
                for h in range(HL):
                    nc.tensor.transpose(
                        trq[:, 128 * h:128 * (h + 1)],
                        q_t[:, DK * h:DK * (h + 1)], ident)
                nc.vector.tensor_copy(
                    qaug[0:64, :, 128 * t:128 * (t + 1)],
                    trq.rearrange("p (h c) -> p h c", h=HL))
                # v cast f32 -> f32r on DVE while we are at it
                v_t = qkin.tile([128, HL * DK], F32, tag="qk")
                nc.sync.dma_start(out=v_t, in_=vd.ap()[128 * t:128 * (t + 1), :])
                nc.vector.tensor_copy(v_r[:, t, :], v_t)

            def p1_block(h, i, rsh):
                e_t = epool.tile([128, L], F32, tag="e")
                for half in range(2):
                    s_ps = spool.tile([128, 1024], F32, tag="s")
                    for c2 in range(2):
                        nc.tensor.matmul(
                            s_ps[:, 512 * c2:512 * (c2 + 1)],
                            qaug[0:64, h, 128 * i:128 * (i + 1)],
                            kaug[0:64, h,
                                 1024 * half + 512 * c2:
                                 1024 * half + 512 * (c2 + 1)],
                            start=True, stop=True)
                    nc.scalar.activation(
                        e_t[:, 1024 * half:1024 * (half + 1)], s_ps,
                        Exp, scale=0.125,
                        accum_out=rsh[:, 2 * i + half:2 * i + half + 1])
                rs_i = statp.tile([128, 1], F32, tag="rs")
                nc.vector.tensor_add(
                    rs_i, rsh[:, 2 * i:2 * i + 1], rsh[:, 2 * i + 1:2 * i + 2])
                recip_i = statp.tile([128, 1], F32, tag="recip")
                nc.vector.reciprocal(recip_i, rs_i)
                nc.vector.tensor_scalar_mul(e_t, e_t, recip_i)
                nc.sync.dma_start(
                    out=attnd.ap()[h, 128 * i:128 * (i + 1), :], in_=e_t)

            def lnz_fill(h, rsh):
                rs16 = statp.tile([128, NB], F32, tag="rs16")
                rshv = rsh.rearrange("p (i two) -> p i two", two=2)
                nc.vector.tensor_add(rs16, rshv[:, :, 0], rshv[:, :, 1])
                ln16 = statp.tile([128, NB], F32, tag="ln16")
                nc.scalar.activation(ln16, rs16, Ln)
                m8 = statp.tile([128, NB], F32, tag="m8")
                nc.vector.tensor_scalar_mul(m8, ln16, -8.0)
                trz = spool.tile([16, 128], F32, tag="s")
                nc.tensor.transpose(trz, m8, ident)
                trzs = statp.tile([16, 128], F32R, tag="trzs")
                nc.vector.tensor_copy(trzs, trz)
                nc.gpsimd.dma_start(
                    out=qaug[64:65, h, :].rearrange("o (j p) -> o j p", j=16),
                    in_=trzs)

            def p2h_block(h, qh, j, pv_q):
                at_t = atpool.tile([128, 1024], F32R, tag="at")
                s2 = spool.tile([128, 1024], F32, tag="s")
                for c2 in range(2):
                    nc.tensor.matmul(
                        s2[:, 512 * c2:512 * (c2 + 1)],
                        kaug[:, h, 128 * j:128 * (j + 1)],
                        qaug[:, h,
                             1024 * qh + 512 * c2:1024 * qh + 512 * (c2 + 1)],
                        start=True, stop=True)
                nc.scalar.activation(at_t, s2, Exp, scale=0.125)
                for c2 in range(2):
                    nc.tensor.matmul(
                        pv_q[c2][:, :],
                        v_r[:, j, DK * h:DK * (h + 1)],
                        at_t[:, 512 * c2:512 * (c2 + 1)],
                        start=(j == 0), stop=(j == NB - 1),
                        skip_group_check=True)

            def fc_block(w, i2):
                i = 8 * w + i2
                y_ps = spool.tile([128, DM], F32, tag="s")
                for n2 in range(2):
                    for h4 in range(HL):
                        nc.tensor.matmul(
                            y_ps[:, 512 * n2:512 * (n2 + 1)],
                            outT[:, h4, 128 * i:128 * (i + 1)],
                            w4[:, h4, 512 * n2:512 * (n2 + 1)],
                            start=(h4 == 0), stop=(h4 == HL - 1))
                ytmp = ypool.tile([128, DM], F32, tag="y")
                nc.vector.tensor_copy(ytmp, y_ps)
                nc.sync.dma_start(
                    out=y_bs[w].ap()[128 * i2:128 * (i2 + 1), :], in_=ytmp)

            # ---------------- pipelined phases ----------------
            # phase h: pass-1 of head h interleaved with pass-2 of head h-1.
            # phase 0 also interleaves the q transposes / v casts.
            # phase 4 runs pass-2 of head 3 with the fc waves + RS folded in.
            rshs = {}
            pvs = {}
            for phase in range(HL + 1):
                h1 = phase
                h2 = phase - 1
                if h1 < HL:
                    rshs[h1] = statp.tile(
                        [128, 2 * NB], F32, tag="rsh", name=f"rsh{h1}")
                for step in range(NB):
                    if phase == 0:
                        q_setup_step(step)
                    if h1 < HL:
                        p1_block(h1, step, rshs[h1])
                    if h2 >= 0:
                        # two pass-2 half-blocks per step
                        for s2i in range(2):
                            idx = 2 * step + s2i
                            qh, j = idx // NB, idx % NB
                            if j == 0:
                                pvs[(h2, qh)] = [
                                    pvpool.tile([64, 512], F32, tag="pv",
                                                name=f"pv{h2}_{qh}_{c2}")
                                    for c2 in range(2)]
                            p2h_block(h2, qh, j, pvs[(h2, qh)])
                            if qh == 1 and j % 2 == 1 and h2 == HL - 1:
                                fc_block(0, j // 2)
                            if j == NB - 1:
                                for c2 in range(2):
                                    nc.vector.tensor_copy(
                                        outT[:, h2,
                                             1024 * qh + 512 * c2:
                                             1024 * qh + 512 * (c2 + 1)],
                                        pvs[(h2, qh)][c2])
                if h1 < HL:
                    lnz_fill(h1, rshs[h1])

            # wave 0 reduce-scatter (wave-0 fc was folded into phase 4)
            nc.gpsimd.collective_compute(
                "ReduceScatter", mybir.AluOpType.add,
                ins=[y_bs[0].ap().opt()], outs=[yr_bs[0].ap().opt()],
                replica_groups=GROUPS)

            for i2 in range(8):
                fc_block(1, i2)
            nc.gpsimd.collective_compute(
                "ReduceScatter", mybir.AluOpType.add,
                ins=[y_bs[1].ap().opt()], outs=[yr_bs[1].ap().opt()],
                replica_groups=GROUPS)

            # ---------------- residual + LayerNorm (two waves) -------------
            for w in range(2):
                for t2 in range(2):
                    yt = ypool.tile([128, DM], F32, tag="y")
                    nc.sync.dma_start(
                        out=yt, in_=yr_bs[w].ap()[128 * t2:128 * (t2 + 1), :])
                    nc.vector.tensor_add(yt, yt, resid_sb[:, 2 * w + t2, :])
                    nc.vector.tensor_add(yt, yt, bfc_t)
                    stats = statp.tile([128, 2, 6], F32, tag="bn")
                    ytv = yt.rearrange("p (s f) -> p s f", s=2)
                    for s in range(2):
                        nc.vector.bn_stats(out=stats[:, s, :], in_=ytv[:, s, :])
                    mv = statp.tile([128, 2], F32, tag="mv")
                    nc.vector.bn_aggr(out=mv, in_=stats)
                    sd = statp.tile([128, 1], F32, tag="sd")
                    nc.scalar.activation(sd, mv[:, 1:2], Sqrt, bias=eps_t)
                    rstd = statp.tile([128, 1], F32, tag="rstd")
                    nc.vector.reciprocal(rstd, sd)
                    nc.vector.tensor_scalar(
                        out=yt, in0=yt, scalar1=mv[:, 0:1], scalar2=rstd,
                        op0=mybir.AluOpType.subtract, op1=mybir.AluOpType.mult)
                    nc.vector.tensor_mul(yt, yt, gamma_t)
                    nc.vector.tensor_add(yt, yt, beta_t)
                    nc.sync.dma_start(
                        out=outd.ap()[128 * (2 * w + t2):
                                      128 * (2 * w + t2 + 1), :], in_=yt)

    _split_multi_waits(nc)
    return nc


_NC = None


def _get_nc():
    global _NC
    if _NC is None:
        _NC = _build()
    return _NC


def kernel(q, k, v, W_fc, b_fc, ln_gamma, ln_beta, _trace=False, _trace_kwargs=None):
    q = np.ascontiguousarray(q, np.float32)
    k = np.ascontiguousarray(k, np.float32)
    v = np.ascontiguousarray(v, np.float32)
    W_fc = np.ascontiguousarray(W_fc, np.float32)
    b_fc = np.ascontiguousarray(b_fc, np.float32).reshape(1, DM)
    ln_gamma = np.ascontiguousarray(ln_gamma, np.float32).reshape(1, DM)
    ln_beta = np.ascontiguousarray(ln_beta, np.float32).reshape(1, DM)

    in_maps = []
    for c in range(8):
        b, r = c // 4, c % 4
        # rows this core owns after the two wave reduce-scatters
        rows = np.r_[256 * r:256 * (r + 1), 1024 + 256 * r:1024 + 256 * (r + 1)]
        in_maps.append({
            "q": q[b][:, 256 * r:256 * (r + 1)],
            "k": k[b][:, 256 * r:256 * (r + 1)],
            "v": v[b][:, 256 * r:256 * (r + 1)],
            "w": W_fc[256 * r:256 * (r + 1), :],
            "resid": np.ascontiguousarray(q[b][rows, :]),
            "bfc": b_fc, "gamma": ln_gamma, "beta": ln_beta,
        })

    nc = _get_nc()
    res = run_bass_kernel_spmd(
        nc, in_maps, core_ids=list(range(8)),
        trace=_trace, **(_trace_kwargs or {}))

    out = np.empty((B, L, DM), np.float32)
    attn = np.empty((B, H, L, L), np.float32)
    for c in range(8):
        b, r = c // 4, c % 4
        rc = res.results[c]
        attn[b, 4 * r:4 * (r + 1)] = rc["attn"]
        out[b, 256 * r:256 * (r + 1)] = rc["out"][0:256]
        out[b, 1024 + 256 * r:1024 + 256 * (r + 1)] = rc["out"][256:512]
    kernel.last_exec_ns = res.exec_time_ns
    return out, attn
